# revision 1
# baseline (speedup 1.0000x reference)
"""Trainium2 Bass kernel for nn_CrossAttention (B=2, C=512, N=M=2048, H=8).

Sharding: batch*heads = 16 (b,h) pairs across 8 cores, 2 heads per core.
Cores 0-3 handle batch 0 (heads 0..7 in pairs), cores 4-7 batch 1.

Per-core math (all matmuls fp32r = tf32-like, full PE rate at free>=256):
  qT[d,n] = (Wq_cols * SCALE).T @ x_b          (2 heads packed on partitions)
  kT[d,m] = Wk_cols.T @ y_b
  vT[d,m] = (Wv_cols * (1+lw)).T @ y_b   -> PE-transpose -> v2[m, d|1] tiles
  S^T[m,n] = kT_h.T-slices @ qT_h        (row-packed K=64 pairs per head)
  P = exp(S^T)                            (ScalarE, streaming blocks)
  [attnT | den] = [v2_h | ones].T @ P     (M=65 ones-augmented, accum over m)
  attnT_norm = attnT * (1/den)            (gpsimd partition-broadcast + DVE)
  outT_partial[c,n] = Wp_rows.T @ attnT_norm

The depthwise conv (ksize=1) folds into Wv scaling + a host-side output bias
(bias' = bp + lb @ Wp, exact because softmax rows sum to 1).
Host sums the 4 per-batch partials and adds bias'.
"""

import os
import sys
import numpy as np
from contextlib import ExitStack

for _p in ("/root/.axon_site", "/root/.axon_site/_ro/trn_rl_repo",
           "/root/.axon_site/_ro/pypackages", "/opt/trn_rl_repo"):
    if os.path.isdir(_p) and _p not in sys.path:
        sys.path.append(_p)

B, C, N, M, H = 2, 512, 2048, 2048, 8
HD = C // H
SCALE = HD ** -0.5
NCORES = 8

_NC = None
LAST_RUN = None


def to_fp32r(x: np.ndarray) -> np.ndarray:
    """Round fp32 to the 20-bit (1s/8e/11m) fp32r grid, round-to-nearest-even."""
    b = np.ascontiguousarray(x, np.float32).view(np.uint32).astype(np.uint64)
    rb = (b >> 12) & 1
    b = (b + 0x7FF + rb) & 0xFFFFF000
    return b.astype(np.uint32).view(np.float32)


def _build_program(reps=1):
    from concourse import bacc
    import concourse.tile as tile
    import concourse.mybir as mybir
    from concourse.masks import make_identity

    F32 = mybir.dt.float32
    F32R = mybir.dt.float32r
    EXP = mybir.ActivationFunctionType.Exp
    MULT = mybir.AluOpType.mult

    nc = bacc.Bacc("TRN2", target_bir_lowering=False, debug=False,
                   num_devices=NCORES)

    xr = nc.dram_tensor("xr", [C, N], F32R, kind="ExternalInput").ap()
    yr = nc.dram_tensor("yr", [C, M], F32R, kind="ExternalInput").ap()
    wq_d = nc.dram_tensor("wq", [C, 128], F32R, kind="ExternalInput").ap()
    wk_d = nc.dram_tensor("wk", [C, 128], F32R, kind="ExternalInput").ap()
    wv_d = nc.dram_tensor("wv", [C, 128], F32R, kind="ExternalInput").ap()
    wp_d = nc.dram_tensor("wp", [128, C], F32R, kind="ExternalInput").ap()
    ones_d = nc.dram_tensor("ones_d", [128, 1], F32R, kind="ExternalInput").ap()
    outT = nc.dram_tensor("outT", [C, N], F32, kind="ExternalOutput").ap()

    with tile.TileContext(nc) as tc, ExitStack() as ctx:
        sb = ctx.enter_context(tc.tile_pool(name="sb", bufs=1))
        ppool = ctx.enter_context(tc.tile_pool(name="ppool", bufs=4))
        npool = ctx.enter_context(tc.tile_pool(name="npool", bufs=2))
        spool = ctx.enter_context(tc.tile_pool(name="spool", bufs=2))
        # PSUM budget (8 banks): psA "blk" 3x[128,1024] = 6 banks (score
        # ring, also proj accumulators / transposes / outproj transients);
        # psB "acc" 2x[65,512] = 2 banks (attn accumulators). Ring depth 3
        # decouples PE from ScalarE's exp stream.
        psA = ctx.enter_context(tc.tile_pool(name="psA", bufs=3, space="PSUM"))
        psB = ctx.enter_context(tc.tile_pool(name="psB", bufs=2, space="PSUM"))

        # ---- constants / weights ----
        ident = sb.tile([128, 128], F32, tag="ident")
        make_identity(nc, ident)
        ones_sb = sb.tile([128, 1], F32R, tag="ones_sb")
        nc.sync.dma_start(out=ones_sb, in_=ones_d)
        # warm the exp table while DMAs stream
        warm = sb.tile([1, 32], F32, tag="warm")
        nc.scalar.activation(warm, ident[0:1, 0:32], EXP)
        # warm the PE clock (HAM) with dummy matmuls so the first
        # projections run at 2.4GHz; transposes don't count as PE-busy.
        psw = psB.tile([128, 128], F32, tag="acc", name="psw")
        for _ in range(8):
            nc.tensor.matmul(psw, ident, ident, start=True, stop=True)
        warm2 = sb.tile([128, 128], F32, tag="warm2")
        nc.vector.tensor_copy(warm2, psw)

        wk_sb = sb.tile([128, 4, 128], F32R, tag="wk_sb")
        wv_sb = sb.tile([128, 4, 128], F32R, tag="wv_sb")
        wq_sb = sb.tile([128, 4, 128], F32R, tag="wq_sb")
        wp_sb = sb.tile([128, C], F32R, tag="wp_sb")

        for rep in range(reps):
            r = f"r{rep}_" if reps > 1 else ""

            # ---- column-sliced input loads on the sync-engine HWDGE ----
            y_sb = [sb.tile([128, M], F32R, tag=f"y_sb{k}", name=f"{r}y_sb{k}")
                    for k in range(4)]
            x_sb = [sb.tile([128, N], F32R, tag=f"x_sb{k}", name=f"{r}x_sb{k}")
                    for k in range(4)]
            # DMA order = consumption order: y j0/j1 gate the prologue
            # projections, x j0 gates qT j0, the rest streams under the
            # main loop (j2/j3 projections are woven into n-chunk 0).
            def load_slices(dst_tiles, src, j):
                js = slice(j * 512, (j + 1) * 512)
                for k in range(4):
                    nc.sync.dma_start(
                        out=dst_tiles[k][:, js],
                        in_=src[k * 128:(k + 1) * 128, js])

            if rep == 0:
                nc.sync.dma_start(
                    out=wk_sb, in_=wk_d.rearrange("(kc p) m -> p kc m", p=128))
            load_slices(y_sb, yr, 0)
            if rep == 0:
                nc.sync.dma_start(
                    out=wv_sb, in_=wv_d.rearrange("(kc p) m -> p kc m", p=128))
            load_slices(x_sb, xr, 0)
            if rep == 0:
                nc.sync.dma_start(
                    out=wq_sb, in_=wq_d.rearrange("(kc p) m -> p kc m", p=128))
            load_slices(y_sb, yr, 1)
            load_slices(y_sb, yr, 2)
            load_slices(y_sb, yr, 3)
            if rep == 0:
                nc.sync.dma_start(out=wp_sb, in_=wp_d)
            load_slices(x_sb, xr, 1)
            load_slices(x_sb, xr, 2)
            load_slices(x_sb, xr, 3)

            kT = sb.tile([128, M], F32R, tag="kT", name=f"{r}kT")
            vT = sb.tile([128, M], F32, tag="vT", name=f"{r}vT")
            qT = sb.tile([128, N], F32R, tag="qT", name=f"{r}qT")
            v2a = [None] * 16
            v2b = [None] * 16

            def proj_half(ps_holder, dst, w_sb, src, j, half, name):
                if half == 0:
                    ps_holder[name] = psA.tile([128, 512], F32, tag="blk",
                                               name=name)
                ps = ps_holder[name]
                for kc in (0, 1) if half == 0 else (2, 3):
                    nc.tensor.matmul(ps, w_sb[:, kc, :],
                                     src[kc][:, j * 512:(j + 1) * 512],
                                     start=(kc == 0), stop=(kc == 3))
                if half == 1:
                    nc.vector.tensor_copy(dst[:, j * 512:(j + 1) * 512], ps)

            def transpose_quad(m0):
                # 4 transposes share one PSUM ring slot (4 col-slices)
                t = psA.tile([128, 512], F32, tag="blk", name=f"{r}pst{m0}")
                for i in range(4):
                    m = m0 + i
                    nc.tensor.transpose(t[:, i * 128:(i + 1) * 128],
                                        vT[:, m * 128:(m + 1) * 128], ident)
                for i in range(4):
                    m = m0 + i
                    c = i * 128
                    a_ = sb.tile([128, 65], F32R, tag=f"v2a{m}",
                                 name=f"{r}v2a{m}")
                    nc.vector.tensor_copy(a_[:, 0:64], t[:, c:c + 64])
                    nc.vector.tensor_copy(a_[:, 64:65], ones_sb)
                    b_ = sb.tile([128, 65], F32R, tag=f"v2b{m}",
                                 name=f"{r}v2b{m}")
                    nc.vector.tensor_copy(b_[:, 0:64], t[:, c + 64:c + 128])
                    nc.vector.tensor_copy(b_[:, 64:65], ones_sb)
                    v2a[m] = a_
                    v2b[m] = b_

            hold = {}

            # ---- prologue: only the j0 chain gates the main loop ----
            for half in (0, 1):
                proj_half(hold, kT, wk_sb, y_sb, 0, half, f"{r}psk0")
            for half in (0, 1):
                proj_half(hold, vT, wv_sb, y_sb, 0, half, f"{r}psv0")
            transpose_quad(0)
            for half in (0, 1):
                proj_half(hold, qT, wq_sb, x_sb, 0, half, f"{r}psq0")

            # fill task groups: one group per m-step, woven between score
            # blocks so the PE finishes late projections without starving
            # ScalarE and without blocking the PSUM ring on late DMAs.
            def P(dst, w, src, j, half, name):
                return lambda: proj_half(hold, dst, w, src, j, half, name)

            fills = {
                0: [[P(kT, wk_sb, y_sb, 1, 0, f"{r}psk1"),
                     P(kT, wk_sb, y_sb, 1, 1, f"{r}psk1")],
                    [P(vT, wv_sb, y_sb, 1, 0, f"{r}psv1"),
                     P(vT, wv_sb, y_sb, 1, 1, f"{r}psv1")],
                    [lambda: transpose_quad(4)],
                    [P(kT, wk_sb, y_sb, 2, 0, f"{r}psk2")],
                    [P(kT, wk_sb, y_sb, 2, 1, f"{r}psk2")],
                    [P(vT, wv_sb, y_sb, 2, 0, f"{r}psv2")],
                    [P(vT, wv_sb, y_sb, 2, 1, f"{r}psv2")],
                    [lambda: transpose_quad(8)],
                    [P(kT, wk_sb, y_sb, 3, 0, f"{r}psk3")],
                    [P(kT, wk_sb, y_sb, 3, 1, f"{r}psk3")],
                    [P(vT, wv_sb, y_sb, 3, 0, f"{r}psv3")],
                    [P(vT, wv_sb, y_sb, 3, 1, f"{r}psv3")],
                    [lambda: transpose_quad(12)],
                    [P(qT, wq_sb, x_sb, 1, 0, f"{r}psq1"),
                     P(qT, wq_sb, x_sb, 1, 1, f"{r}psq1")]],
                1: [[P(qT, wq_sb, x_sb, 2, 0, f"{r}psq2"),
                     P(qT, wq_sb, x_sb, 2, 1, f"{r}psq2")]],
                2: [[P(qT, wq_sb, x_sb, 3, 0, f"{r}psq3"),
                     P(qT, wq_sb, x_sb, 3, 1, f"{r}psq3")]],
                3: [],
            }

            # ---- attention main loop over the global block stream, with
            # attnout lagging one block behind scores/exp so the PE never
            # serializes attnout(n,15) -> scores(n+1,0) at chunk boundaries.
            ah = {}
            pending_out = None   # (n, nrm) awaiting output projection
            prev = None          # (n, m, P, ah0, ah1) awaiting attnout

            def emit_outproj(po_n, po_nrm, cc):
                po = psA.tile([128, 512], F32, tag="blk",
                              name=f"{r}po{po_n}_{cc}")
                nc.tensor.matmul(po, wp_sb[:, cc * 128:(cc + 1) * 128],
                                 po_nrm, start=True, stop=True)
                so = npool.tile([128, 512], F32, tag="so",
                                name=f"{r}so{po_n}_{cc}")
                nc.vector.tensor_copy(so, po)
                nc.sync.dma_start(
                    out=outT[cc * 128:(cc + 1) * 128,
                             po_n * 512:(po_n + 1) * 512],
                    in_=so)

            def emit_attnout(pn, pm, pP, pah0, pah1):
                nonlocal pending_out
                nc.tensor.matmul(pah0, v2a[pm], pP[:, 0:512],
                                 start=(pm == 0), stop=(pm == 15))
                nc.tensor.matmul(pah1, v2b[pm], pP[:, 512:1024],
                                 start=(pm == 0), stop=(pm == 15))
                if pm == 15:
                    # normalize attnT / den (den = row 64); overlaps the
                    # next n-chunk's score/exp stream on DVE+Pool.
                    nrm = npool.tile([128, 512], F32R, tag="nrm",
                                     name=f"{r}nrm{pn}")
                    for hi, a in ((0, pah0), (1, pah1)):
                        rd = spool.tile([1, 512], F32, tag=f"rd{hi}",
                                        name=f"{r}rd{hi}_{pn}")
                        nc.vector.reciprocal(rd, a[64:65, :])
                        rb = spool.tile([64, 512], F32, tag=f"rb{hi}",
                                        name=f"{r}rb{hi}_{pn}")
                        nc.gpsimd.partition_broadcast(rb, rd)
                        nc.vector.tensor_tensor(nrm[hi * 64:(hi + 1) * 64, :],
                                                a[0:64, :], rb, op=MULT)
                    pending_out = (pn, nrm)

            for n in range(4):
                ns = slice(n * 512, (n + 1) * 512)
                ah0 = psB.tile([65, 512], F32, tag="acc", name=f"{r}ah0_{n}")
                ah1 = psB.tile([65, 512], F32, tag="acc", name=f"{r}ah1_{n}")
                for m in range(16):
                    ms = slice(m * 128, (m + 1) * 128)
                    blk = psA.tile([128, 1024], F32, tag="blk",
                                   name=f"{r}blk{n}_{m}")
                    nc.tensor.matmul(blk[:, 0:512], kT[0:64, ms], qT[0:64, ns],
                                     start=True, stop=True, tile_position=(0, 0))
                    nc.tensor.matmul(blk[:, 512:1024], kT[64:128, ms],
                                     qT[64:128, ns],
                                     start=True, stop=True, tile_position=(64, 0))
                    P = ppool.tile([128, 1024], F32R, tag="p", name=f"{r}p{n}_{m}")
                    nc.scalar.activation(P, blk, EXP)
                    if m >= 1 and fills[n]:
                        for task in fills[n].pop(0):
                            task()
                    if prev is not None:
                        emit_attnout(*prev)
                    prev = (n, m, P, ah0, ah1)
                    if pending_out is not None and m in (3, 6, 9, 12):
                        po_n, po_nrm = pending_out
                        emit_outproj(po_n, po_nrm, (m - 3) // 3)
            # drain the lagged block, then the last n-chunk's outproj
            emit_attnout(*prev)
            po_n, po_nrm = pending_out
            for cc in range(4):
                emit_outproj(po_n, po_nrm, cc)

    nc.compile()
    return nc


def _get_program():
    global _NC
    if _NC is None:
        _NC = _build_program()
    return _NC


def make_in_maps(inputs):
    x = np.asarray(inputs["x"], np.float32)
    y = np.asarray(inputs["y"], np.float32)
    Wq = np.asarray(inputs["Wq"], np.float32)
    Wkv = np.asarray(inputs["Wkv"], np.float32)
    lw = np.asarray(inputs["lw"], np.float32)

    d = np.arange(HD)
    ones = np.ones((128, 1), np.float32)
    xr = [to_fp32r(x[b]) for b in range(B)]
    yr = [to_fp32r(y[b]) for b in range(B)]
    in_maps = []
    for core in range(NCORES):
        b = core // 4
        h0 = (core % 4) * 2
        ch = np.concatenate([h * HD + d for h in (h0, h0 + 1)])  # channels
        colsK = np.concatenate([h * 2 * HD + 2 * d for h in (h0, h0 + 1)])
        wq_c = Wq[:, ch] * np.float32(SCALE)
        wk_c = Wkv[:, colsK]
        wv_c = Wkv[:, colsK + 1] * (1.0 + lw[ch])[None, :]
        wp_c = np.asarray(inputs["Wp"], np.float32)[ch, :]
        in_maps.append({
            "xr": xr[b],
            "yr": yr[b],
            "wq": to_fp32r(wq_c),
            "wk": to_fp32r(wk_c),
            "wv": to_fp32r(wv_c),
            "wp": to_fp32r(wp_c),
            "ones_d": ones,
        })
    return in_maps


def assemble_output(results, inputs):
    lb = np.asarray(inputs["lb"], np.float32)
    Wp = np.asarray(inputs["Wp"], np.float32)
    bp = np.asarray(inputs["bp"], np.float32)
    bias = (bp + lb @ Wp).astype(np.float32)
    out = np.stack([
        results[0]["outT"] + results[1]["outT"]
        + results[2]["outT"] + results[3]["outT"],
        results[4]["outT"] + results[5]["outT"]
        + results[6]["outT"] + results[7]["outT"],
    ])
    out += bias[None, :, None]
    return out.astype(np.float32)


def kernel(x, y, Wq, Wkv, lw, lb, Wp, bp):
    global LAST_RUN
    from concourse.bass_utils import run_bass_kernel_spmd

    inputs = dict(x=x, y=y, Wq=Wq, Wkv=Wkv, lw=lw, lb=lb, Wp=Wp, bp=bp)
    nc = _get_program()
    in_maps = make_in_maps(inputs)
    LAST_RUN = run_bass_kernel_spmd(nc, in_maps, list(range(NCORES)))
    return assemble_output(LAST_RUN.results, inputs)



# revision 6
# speedup vs baseline: 1.1448x; 1.1448x over previous
"""Trainium2 Bass kernel for nn_CrossAttention (B=2, C=512, N=M=2048, H=8).

Sharding: batch*heads = 16 (b,h) pairs across 8 cores, 2 heads per core.
Cores 0-3 handle batch 0 (heads 0..7 in pairs), cores 4-7 batch 1.

Cost-model-driven design (matmul cost = moving rows only; fp16 = 1 c/r):
  qT[d,n] = (Wq_cols*SCALE).T @ x_b   fp16, 4 kc-chunks, psum->sbuf fp16
  kT[d,m] = Wk_cols.T @ y_b           fp16
  v2[m,d] = y_b_cols.T @ Wv'          FLIPPED projection (y stationary)
                                      -> [m,d] layout directly, no transposes
  S^T[m,n] = kT_h.T @ qT_h            quadrant-packed pairs, psum [128,1024]
  P = exp(S^T) fp16                   ScalarE exact exp for most blocks;
                                      DVE fast-exp (int16 bitcast) for some
  A[n,d] += P_chunk.T @ v2            FLIPPED attn-out: stationary P
                                      [128m,128n] fp16, moving v2 [128m,64]
                                      (ap=64); den[n] += P_chunk.T @ ones
  A_norm = A * (1/den)                released to SBUF, per-partition scalar
                                      on gpsimd (SBUF-only engine)
  attnT[d,n] = PE-transpose(A_norm)   fp16 transposes into psum-bitcast
  outT_part[c,n] = Wp_rows.T @ attnT  psum f32 -> sbuf -> DMA

PSUM (8 banks): psS ring 3x[128,1024] (6) + accA [128,512] (1, released
via one DVE copy per chunk) + dens [128,16] (1, parity-shared).
PSUM start=True lazily zeroes the whole 2KB bank -> exactly one start per
bank per accumulation epoch; later writes self-initialize via pending-zero.

Depthwise conv (ksize=1) folds into Wv scaling + host-side output bias
(bias' = bp + lb @ Wp, exact because softmax rows sum to 1).
Host sums the 4 per-batch partials and adds bias'.
"""

import os
import sys
import numpy as np
from contextlib import ExitStack

for _p in ("/root/.axon_site", "/root/.axon_site/_ro/trn_rl_repo",
           "/root/.axon_site/_ro/pypackages", "/opt/trn_rl_repo"):
    if os.path.isdir(_p) and _p not in sys.path:
        sys.path.append(_p)

B, C, N, M, H = 2, 512, 2048, 2048, 8
HD = C // H
SCALE = HD ** -0.5
NCORES = 8

# m-steps whose exp runs as DVE fast-exp (rest: ScalarE exact exp)
DVE_EXP_M = (5, 9, 13)
WARM_N = 16

_NC = None
LAST_RUN = None

LOG2E = 1.4426950408889634
FE_SCALE = 1024.0 * LOG2E
FE_BIAS = 15.0 * 1024.0 - 44.65


def _build_program(reps=1):
    from concourse import bacc
    import concourse.tile as tile
    import concourse.mybir as mybir
    from concourse.masks import make_identity

    F32 = mybir.dt.float32
    F16 = mybir.dt.float16
    I16 = mybir.dt.int16
    EXP = mybir.ActivationFunctionType.Exp
    MULT = mybir.AluOpType.mult
    ADD = mybir.AluOpType.add

    nc = bacc.Bacc("TRN2", target_bir_lowering=False, debug=False,
                   num_devices=NCORES)

    xr = nc.dram_tensor("xr", [C, N], F16, kind="ExternalInput").ap()
    yr = nc.dram_tensor("yr", [C, M], F16, kind="ExternalInput").ap()
    wq_d = nc.dram_tensor("wq", [C, 128], F16, kind="ExternalInput").ap()
    wk_d = nc.dram_tensor("wk", [C, 128], F16, kind="ExternalInput").ap()
    wv_d = nc.dram_tensor("wv", [C, 128], F16, kind="ExternalInput").ap()
    wp_d = nc.dram_tensor("wp", [128, C], F16, kind="ExternalInput").ap()
    outT = nc.dram_tensor("outT", [C, N], F32, kind="ExternalOutput").ap()

    with tile.TileContext(nc) as tc, ExitStack() as ctx:
        sb = ctx.enter_context(tc.tile_pool(name="sb", bufs=1))
        ppool = ctx.enter_context(tc.tile_pool(name="ppool", bufs=4))
        apool = ctx.enter_context(tc.tile_pool(name="apool", bufs=2))
        npool = ctx.enter_context(tc.tile_pool(name="npool", bufs=2))
        psS = ctx.enter_context(tc.tile_pool(name="psS", bufs=3, space="PSUM"))
        psAcc = ctx.enter_context(tc.tile_pool(name="psAcc", bufs=1,
                                               space="PSUM"))
        psDen = ctx.enter_context(tc.tile_pool(name="psDen", bufs=1,
                                               space="PSUM"))

        # ---- constants / weights ----
        ident16 = sb.tile([128, 128], F16, tag="ident16")
        make_identity(nc, ident16)
        ones1 = sb.tile([128, 1], F16, tag="ones1")
        nc.gpsimd.memset(ones1, 1.0)
        # warm the exp table while DMAs stream
        warm = sb.tile([1, 32], F32, tag="warm")
        nc.scalar.activation(warm, ident16[0:1, 0:32], EXP)
        # warm the PE clock (p-state) with dummy matmuls under the DMA wait
        psw = psS.tile([128, 1024], F32, tag="blk", name="psw")
        for _ in range(WARM_N):
            nc.tensor.matmul(psw[:, 0:128], ident16, ident16,
                             start=True, stop=True)
        warm2 = sb.tile([128, 128], F32, tag="warm2")
        nc.vector.tensor_copy(warm2, psw[:, 0:128])

        wk_sb = sb.tile([128, 4, 128], F16, tag="wk_sb")
        wv_sb = sb.tile([128, 4, 128], F16, tag="wv_sb")
        wq_sb = sb.tile([128, 4, 128], F16, tag="wq_sb")
        wp_sb = sb.tile([128, C], F16, tag="wp_sb")

        acc = psAcc.tile([128, 512], F32, tag="accA", name="acc")
        dens = psDen.tile([128, 16], F32, tag="dens", name="dens")

        for rep in range(reps):
            r = f"r{rep}_" if reps > 1 else ""

            y_all = sb.tile([128, 4, M], F16, tag="y_all", name=f"{r}y_all")
            x_all = sb.tile([128, 4, N], F16, tag="x_all", name=f"{r}x_all")
            yre = yr.rearrange("(kc p) m -> p kc m", p=128)
            xre = xr.rearrange("(kc p) m -> p kc m", p=128)

            def load_j(dst, srcre, j):
                js = slice(j * 512, (j + 1) * 512)
                nc.sync.dma_start(out=dst[:, :, js], in_=srcre[:, :, js])

            if rep == 0:
                nc.sync.dma_start(
                    out=wk_sb, in_=wk_d.rearrange("(kc p) m -> p kc m", p=128))
                nc.sync.dma_start(
                    out=wv_sb, in_=wv_d.rearrange("(kc p) m -> p kc m", p=128))
                nc.sync.dma_start(
                    out=wq_sb, in_=wq_d.rearrange("(kc p) m -> p kc m", p=128))
            load_j(y_all, yre, 0)
            load_j(x_all, xre, 0)
            if rep == 0:
                nc.sync.dma_start(out=wp_sb, in_=wp_d)
            load_j(y_all, yre, 1)
            load_j(y_all, yre, 2)
            load_j(y_all, yre, 3)
            load_j(x_all, xre, 1)
            load_j(x_all, xre, 2)
            load_j(x_all, xre, 3)

            kT = sb.tile([128, M], F16, tag="kT", name=f"{r}kT")
            qT = sb.tile([128, N], F16, tag="qT", name=f"{r}qT")
            # v2_all[:, m*128 + h*64 : +64] = moving v slice for (m-block, head)
            v2_all = sb.tile([128, 16 * 128], F16, tag="v2_all",
                             name=f"{r}v2_all")

            # ---- projections ----
            def kq_proj(dst, w_sb, src, j, half, hold, name):
                # unflipped: stationary w chunk [128,128], moving src [128,512]
                if half == 0:
                    hold[name] = psS.tile([128, 1024], F32, tag="blk",
                                          name=name)
                ps = hold[name]
                for kc in (0, 1) if half == 0 else (2, 3):
                    nc.tensor.matmul(ps[:, 0:512], w_sb[:, kc, :],
                                     src[:, kc, j * 512:(j + 1) * 512],
                                     start=(kc == 0), stop=(kc == 3))
                if half == 1:
                    nc.vector.tensor_copy(dst[:, j * 512:(j + 1) * 512],
                                          ps[:, 0:512])

            def v_proj(g):
                # flipped: stationary y chunk [128c,128m], moving wv [128c,128d]
                # 4 m-blocks (g*4 .. g*4+3) -> one psum [128,512] -> one copy
                ps = psS.tile([128, 1024], F32, tag="blk", name=f"{r}psv{g}")
                for i in range(4):
                    mblk = g * 4 + i
                    ms = slice(mblk * 128, (mblk + 1) * 128)
                    for kc in range(4):
                        nc.tensor.matmul(
                            ps[:, i * 128:(i + 1) * 128],
                            y_all[:, kc, ms], wv_sb[:, kc, :],
                            start=(kc == 0), stop=(kc == 3))
                nc.vector.tensor_copy(
                    v2_all[:, g * 512:(g + 1) * 512], ps[:, 0:512])

            hold = {}

            # ---- prologue: j0 projections gate chunk 0 (v_proj(0) is only
            # needed by the first attn-out at m=1 -> goes into fills) ----
            for half in (0, 1):
                kq_proj(kT, wk_sb, y_all, 0, half, hold, f"{r}psk0")
            for half in (0, 1):
                kq_proj(qT, wq_sb, x_all, 0, half, hold, f"{r}psq0")

            def K(j, half):
                return lambda: kq_proj(kT, wk_sb, y_all, j, half, hold,
                                       f"{r}psk{j}")

            def Q(j, half):
                return lambda: kq_proj(qT, wq_sb, x_all, j, half, hold,
                                       f"{r}psq{j}")

            def V(g):
                return lambda: v_proj(g)

            fills = {
                0: [K(1, 0), K(1, 1), V(1), K(2, 0), K(2, 1),
                    V(2), K(3, 0), K(3, 1), V(3), Q(1, 0), Q(1, 1)],
                1: [Q(2, 0), Q(2, 1)],
                2: [Q(3, 0), Q(3, 1)],
                3: [],
            }

            # ---- attention main loop ----
            # accA bank: cols s*128+h*64 = A(slice s, head h) [128,64]
            # dens bank: cols (n%2)*8 + s*2 + h = den column of chunk n
            prev = None          # (P, m, n) awaiting attn-out
            pending = []         # normalization chain tasks of chunk n-1

            def emit_attnout(pP, pm, pn):
                first = pm == 0
                dof = (pn % 2) * 8
                for s in range(4):
                    for h in range(2):
                        stat = pP[:, h * 512 + s * 128:h * 512 + (s + 1) * 128]
                        nc.tensor.matmul(
                            acc[:, s * 128 + h * 64:s * 128 + h * 64 + 64],
                            stat,
                            v2_all[:, pm * 128 + h * 64:pm * 128 + h * 64 + 64],
                            start=(first and s == 0 and h == 0),
                            stop=(pm == 15), skip_group_check=True)
                        nc.tensor.matmul(
                            dens[:, dof + s * 2 + h:dof + s * 2 + h + 1],
                            stat, ones1,
                            start=(first and s == 0 and h == 0),
                            stop=(pm == 15), skip_group_check=True)

            def make_pending(n):
                # release-copy acc -> sbuf, then normalize (gpsimd, SBUF-only),
                # transpose, output-project; woven into chunk n+1.
                asb = npool.tile([128, 512], F32, tag="asb", name=f"{r}asb{n}")
                rc = npool.tile([128, 8], F32, tag="rc", name=f"{r}rc{n}")
                apk = [None] * 4
                attnT = npool.tile([128, 512], F16, tag="attnT",
                                   name=f"{r}attnT{n}")
                so = npool.tile([128, 4, 512], F32, tag="so", name=f"{r}so{n}")

                def release():
                    nc.vector.tensor_copy(asb, acc)
                    nc.vector.reciprocal(rc, dens[:, (n % 2) * 8:(n % 2) * 8 + 8])

                def norm(s0):
                    for s in (s0, s0 + 1):
                        t = apool.tile([128, 128], F16, tag=f"apk{s}",
                                       name=f"{r}apk{n}_{s}")
                        for h in range(2):
                            nc.gpsimd.tensor_scalar(
                                t[:, h * 64:(h + 1) * 64],
                                asb[:, s * 128 + h * 64:s * 128 + h * 64 + 64],
                                rc[:, s * 2 + h:s * 2 + h + 1], None, MULT)
                        apk[s] = t

                def trans():
                    ps = psS.tile([128, 1024], F32, tag="blk",
                                  name=f"{r}pst{n}")
                    pv = ps.bitcast(F16)  # [128, 2048] fp16 view
                    for s in range(4):
                        nc.tensor.transpose(pv[:, s * 128:(s + 1) * 128],
                                            apk[s], ident16)
                    for s in range(4):
                        nc.vector.tensor_copy(attnT[:, s * 128:(s + 1) * 128],
                                              pv[:, s * 128:(s + 1) * 128])

                def outproj(cc):
                    po = psS.tile([128, 1024], F32, tag="blk",
                                  name=f"{r}po{n}_{cc}")
                    nc.tensor.matmul(po[:, 0:512],
                                     wp_sb[:, cc * 128:(cc + 1) * 128],
                                     attnT, start=True, stop=True)
                    nc.vector.tensor_copy(so[:, cc, :], po[:, 0:512])
                    if cc == 3:
                        nc.sync.dma_start(
                            out=outT.rearrange("(cc p) n -> p cc n", p=128)[
                                :, :, n * 512:(n + 1) * 512],
                            in_=so)

                return [release, lambda: norm(0), lambda: norm(2), trans,
                        lambda: outproj(0), lambda: outproj(1),
                        lambda: outproj(2), lambda: outproj(3)]

            for n in range(4):
                ns = slice(n * 512, (n + 1) * 512)
                tasks = pending + fills[n]
                pending = []
                for m in range(16):
                    msl = slice(m * 128, (m + 1) * 128)
                    blk = psS.tile([128, 1024], F32, tag="blk",
                                   name=f"{r}blk{n}_{m}")
                    nc.tensor.matmul(blk[:, 0:512], kT[0:64, msl],
                                     qT[0:64, ns], start=True, stop=True,
                                     tile_position=(0, 0))
                    nc.tensor.matmul(blk[:, 512:1024], kT[64:128, msl],
                                     qT[64:128, ns], start=True, stop=True,
                                     tile_position=(64, 0))
                    P = ppool.tile([128, 1024], F16, tag="p",
                                   name=f"{r}p{n}_{m}")
                    if m in DVE_EXP_M:
                        nc.vector.tensor_scalar(P.bitcast(I16), blk,
                                                FE_SCALE, FE_BIAS, MULT, ADD)
                    else:
                        nc.scalar.activation(P, blk, EXP)
                    if m >= 1 and tasks:
                        tasks.pop(0)()
                    if prev is not None:
                        emit_attnout(*prev)
                    prev = (P, m, n)
                pending = make_pending(n)

            # drain: last attn-out + final chunk's normalization chain
            emit_attnout(*prev)
            prev = None
            for task in pending:
                task()

    nc.compile()
    return nc


def _get_program():
    global _NC
    if _NC is None:
        _NC = _build_program()
    return _NC


def make_in_maps(inputs):
    x = np.asarray(inputs["x"], np.float32)
    y = np.asarray(inputs["y"], np.float32)
    Wq = np.asarray(inputs["Wq"], np.float32)
    Wkv = np.asarray(inputs["Wkv"], np.float32)
    lw = np.asarray(inputs["lw"], np.float32)

    d = np.arange(HD)
    xr = [x[b].astype(np.float16) for b in range(B)]
    yr = [y[b].astype(np.float16) for b in range(B)]
    in_maps = []
    for core in range(NCORES):
        b = core // 4
        h0 = (core % 4) * 2
        ch = np.concatenate([h * HD + d for h in (h0, h0 + 1)])  # channels
        colsK = np.concatenate([h * 2 * HD + 2 * d for h in (h0, h0 + 1)])
        wq_c = Wq[:, ch] * np.float32(SCALE)
        wk_c = Wkv[:, colsK]
        wv_c = Wkv[:, colsK + 1] * (1.0 + lw[ch])[None, :]
        wp_c = np.asarray(inputs["Wp"], np.float32)[ch, :]
        in_maps.append({
            "xr": xr[b],
            "yr": yr[b],
            "wq": wq_c.astype(np.float16),
            "wk": wk_c.astype(np.float16),
            "wv": wv_c.astype(np.float16),
            "wp": wp_c.astype(np.float16),
        })
    return in_maps


def assemble_output(results, inputs):
    lb = np.asarray(inputs["lb"], np.float32)
    Wp = np.asarray(inputs["Wp"], np.float32)
    bp = np.asarray(inputs["bp"], np.float32)
    bias = (bp + lb @ Wp).astype(np.float32)
    out = np.stack([
        results[0]["outT"] + results[1]["outT"]
        + results[2]["outT"] + results[3]["outT"],
        results[4]["outT"] + results[5]["outT"]
        + results[6]["outT"] + results[7]["outT"],
    ])
    out += bias[None, :, None]
    return out.astype(np.float32)


def kernel(x, y, Wq, Wkv, lw, lb, Wp, bp):
    global LAST_RUN
    from concourse.bass_utils import run_bass_kernel_spmd

    inputs = dict(x=x, y=y, Wq=Wq, Wkv=Wkv, lw=lw, lb=lb, Wp=Wp, bp=bp)
    nc = _get_program()
    in_maps = make_in_maps(inputs)
    LAST_RUN = run_bass_kernel_spmd(nc, in_maps, list(range(NCORES)))
    return assemble_output(LAST_RUN.results, inputs)


# revision 47
# speedup vs baseline: 1.2987x; 1.1344x over previous
"""Trainium2 Bass kernel for nn_CrossAttention (B=2, C=512, N=M=2048, H=8).

Sharding: batch*heads = 16 (b,h) pairs across 8 cores, 2 heads per core.
Cores 0-3 handle batch 0 (heads 0..7 in pairs), cores 4-7 batch 1.

Cost-model-driven design (matmul cost = moving rows only; fp16 = 1 c/r):
  qT[d,n] = (Wq_cols*SCALE).T @ x_b   fp16, 4 kc-chunks, psum->sbuf fp16
  kT[d,m] = Wk_cols.T @ y_b           fp16
  v2[m,d] = y_b_cols.T @ Wv'          FLIPPED projection (y stationary)
                                      -> [m,d] layout directly, no transposes
  S^T[m,n] = kT_h.T @ qT_h            quadrant-packed pairs, psum [128,1024]
  P = exp(S^T) fp16                   ScalarE exact exp for most blocks;
                                      DVE fast-exp (int16 bitcast) for some
  A[n,d] += P_chunk.T @ v2            FLIPPED attn-out: stationary P
                                      [128m,128n] fp16, moving v2 [128m,64]
                                      (ap=64); den[n] += P_chunk.T @ ones
  A_norm = A * (1/den)                released to SBUF, per-partition scalar
                                      on gpsimd (SBUF-only engine)
  attnT[d,n] = PE-transpose(A_norm)   fp16 transposes into psum-bitcast
  outT_part[c,n] = Wp_rows.T @ attnT  psum f32 -> sbuf -> DMA

PSUM (8 banks): psS ring 3x[128,1024] (6) + accA [128,512] (1, released
via one DVE copy per chunk) + dens [128,16] (1, parity-shared).
PSUM start=True lazily zeroes the whole 2KB bank -> exactly one start per
bank per accumulation epoch; later writes self-initialize via pending-zero.

Depthwise conv (ksize=1) folds into Wv scaling + host-side output bias
(bias' = bp + lb @ Wp, exact because softmax rows sum to 1).
Host sums the 4 per-batch partials and adds bias'.
"""

import os
import sys
import numpy as np
from contextlib import ExitStack

for _p in ("/root/.axon_site", "/root/.axon_site/_ro/trn_rl_repo",
           "/root/.axon_site/_ro/pypackages", "/opt/trn_rl_repo"):
    if os.path.isdir(_p) and _p not in sys.path:
        sys.path.append(_p)

B, C, N, M, H = 2, 512, 2048, 2048, 8
HD = C // H
SCALE = HD ** -0.5
NCORES = 8

# m-steps whose exp runs as DVE fast-exp (rest: ScalarE exact exp)
DVE_EXP_M = {0: (0, 3, 6, 9, 12, 14), 1: (0, 3, 6, 9, 12, 14),
             2: (0, 3, 6, 9, 12, 14), 3: (0, 3, 6, 9, 12, 14)}
WARM_N = 2

_NC = None
LAST_RUN = None

LOG2E = 1.4426950408889634
FE_SCALE = 1024.0 * LOG2E
FE_BIAS = 15.0 * 1024.0 - 66.85


def _build_program(reps=1):
    from concourse import bacc
    import concourse.tile as tile
    import concourse.mybir as mybir
    from concourse.masks import make_identity

    F32 = mybir.dt.float32
    F16 = mybir.dt.float16
    I16 = mybir.dt.int16
    EXP = mybir.ActivationFunctionType.Exp
    COPY = mybir.ActivationFunctionType.Copy
    MULT = mybir.AluOpType.mult
    ADD = mybir.AluOpType.add

    nc = bacc.Bacc("TRN2", target_bir_lowering=False, debug=False,
                   num_devices=NCORES)

    xr = nc.dram_tensor("xr", [C, N], F16, kind="ExternalInput").ap()
    yr = nc.dram_tensor("yr", [C, M], F16, kind="ExternalInput").ap()
    wq_d = nc.dram_tensor("wq", [C, 128], F16, kind="ExternalInput").ap()
    wk_d = nc.dram_tensor("wk", [C, 128], F16, kind="ExternalInput").ap()
    wv_d = nc.dram_tensor("wv", [C, 128], F16, kind="ExternalInput").ap()
    wp_d = nc.dram_tensor("wp", [128, C], F16, kind="ExternalInput").ap()
    outT = nc.dram_tensor("outT", [C, N], F16, kind="ExternalOutput").ap()

    with tile.TileContext(nc) as tc, ExitStack() as ctx:
        sb = ctx.enter_context(tc.tile_pool(name="sb", bufs=1))
        ppool = ctx.enter_context(tc.tile_pool(name="ppool", bufs=8))
        apool = ctx.enter_context(tc.tile_pool(name="apool", bufs=4))
        npool = ctx.enter_context(tc.tile_pool(name="npool", bufs=4))
        psS = ctx.enter_context(tc.tile_pool(name="psS", bufs=3, space="PSUM"))
        psAcc = ctx.enter_context(tc.tile_pool(name="psAcc", bufs=1,
                                               space="PSUM"))
        psDen = ctx.enter_context(tc.tile_pool(name="psDen", bufs=1,
                                               space="PSUM"))

        # ---- constants / weights ----
        ident16 = sb.tile([128, 128], F16, tag="ident16")
        make_identity(nc, ident16)
        ones1 = sb.tile([128, 1], F16, tag="ones1")
        nc.gpsimd.memset(ones1, 1.0)
        # warm the exp table while DMAs stream
        warm = sb.tile([1, 32], F32, tag="warm")
        nc.scalar.activation(warm, ident16[0:1, 0:32], EXP)
        # warm the PE clock (p-state) with dummy matmuls under the DMA wait
        pswt = psS.tile([128, 1024], F32, tag="blk", name="psw")
        psw = pswt[:, 0:512]
        for _ in range(WARM_N):
            nc.tensor.matmul(psw[:, 0:128], ident16, ident16,
                             start=True, stop=True)
        warm2 = sb.tile([128, 128], F32, tag="warm2")
        nc.vector.tensor_copy(warm2, psw[:, 0:128])

        wk_sb = sb.tile([128, 4, 128], F16, tag="wk_sb")
        wv_sb = sb.tile([128, 4, 128], F16, tag="wv_sb")
        wq_sb = sb.tile([128, 4, 128], F16, tag="wq_sb")
        wp_sb = sb.tile([128, C], F16, tag="wp_sb")

        acc = psAcc.tile([128, 512], F32, tag="accA", name="acc")
        dens = psDen.tile([128, 16], F32, tag="dens", name="dens")

        for rep in range(reps):
            r = f"r{rep}_" if reps > 1 else ""

            y_all = sb.tile([128, 4, M], F16, tag="y_all", name=f"{r}y_all")
            x_all = sb.tile([128, 4, N], F16, tag="x_all", name=f"{r}x_all")
            yre = yr.rearrange("(kc p) m -> p kc m", p=128)
            xre = xr.rearrange("(kc p) m -> p kc m", p=128)

            def load_j(dst, srcre, j):
                js = slice(j * 512, (j + 1) * 512)
                nc.sync.dma_start(out=dst[:, :, js], in_=srcre[:, :, js])

            nc.sync.dma_start(out=y_all[:, 0:2, 0:512],
                              in_=yre[:, 0:2, 0:512])
            if rep == 0:
                nc.sync.dma_start(
                    out=wk_sb, in_=wk_d.rearrange("(kc p) m -> p kc m", p=128))
            nc.sync.dma_start(out=y_all[:, 2:4, 0:512],
                              in_=yre[:, 2:4, 0:512])
            nc.sync.dma_start(out=x_all[:, 0:2, 0:512],
                              in_=xre[:, 0:2, 0:512])
            nc.sync.dma_start(out=x_all[:, 2:4, 0:512],
                              in_=xre[:, 2:4, 0:512])
            if rep == 0:
                nc.sync.dma_start(
                    out=wq_sb, in_=wq_d.rearrange("(kc p) m -> p kc m", p=128))
            load_j(y_all, yre, 1)
            if rep == 0:
                nc.sync.dma_start(
                    out=wv_sb, in_=wv_d.rearrange("(kc p) m -> p kc m", p=128))
            load_j(y_all, yre, 2)
            load_j(y_all, yre, 3)
            load_j(x_all, xre, 1)
            if rep == 0:
                nc.sync.dma_start(out=wp_sb, in_=wp_d)
            load_j(x_all, xre, 2)
            load_j(x_all, xre, 3)


            kT = sb.tile([128, M], F16, tag="kT", name=f"{r}kT")
            qT = sb.tile([128, N], F16, tag="qT", name=f"{r}qT")
            # v2_all[:, m*128 + h*64 : +64] = moving v slice for (m-block, head)
            v2_all = sb.tile([128, 16 * 128], F16, tag="v2_all",
                             name=f"{r}v2_all")

            # ---- projections ----
            def kq_proj(dst, w_sb, src, j, half, hold, name):
                # unflipped: stationary w chunk [128,128], moving src [128,512]
                if half == 0:
                    hold[name] = psS.tile([128, 1024], F32, tag="blk",
                                          name=name)
                ps = hold[name][:, 0:512]
                for kc in (0, 1) if half == 0 else (2, 3):
                    nc.tensor.matmul(ps, w_sb[:, kc, :],
                                     src[:, kc, j * 512:(j + 1) * 512],
                                     start=(kc == 0), stop=(kc == 3))
                if half == 1:
                    nc.vector.tensor_copy(dst[:, j * 512:(j + 1) * 512], ps)

            def v_proj(g, half, hold):
                # flipped: stationary y chunk [128c,128m], moving wv [128c,128d]
                # 4 m-blocks (g*4 .. g*4+3) -> one psum [128,512] -> one copy
                name = f"{r}psv{g}"
                if half == 0:
                    hold[name] = psS.tile([128, 1024], F32, tag="blk",
                                          name=name)
                ps = hold[name][:, 0:512]
                for i in (0, 1) if half == 0 else (2, 3):
                    mblk = g * 4 + i
                    ms = slice(mblk * 128, (mblk + 1) * 128)
                    for kc in range(4):
                        nc.tensor.matmul(
                            ps[:, i * 128:(i + 1) * 128],
                            y_all[:, kc, ms], wv_sb[:, kc, :],
                            start=(kc == 0), stop=(kc == 3))
                if half == 1:
                    nc.vector.tensor_copy(
                        v2_all[:, g * 512:(g + 1) * 512], ps)

            hold = {}

            # ---- prologue: j0 projections gate chunk 0 (v_proj(0) is only
            # needed by the first attn-out at m=1 -> goes into fills) ----
            for half in (0, 1):
                kq_proj(kT, wk_sb, y_all, 0, half, hold, f"{r}psk0")
            for half in (0, 1):
                kq_proj(qT, wq_sb, x_all, 0, half, hold, f"{r}psq0")

            def K(j, half):
                return lambda: kq_proj(kT, wk_sb, y_all, j, half, hold,
                                       f"{r}psk{j}")

            def Q(j, half):
                return lambda: kq_proj(qT, wq_sb, x_all, j, half, hold,
                                       f"{r}psq{j}")

            def V(g, half):
                return lambda: v_proj(g, half, hold)

            fills = {
                0: [V(1, 0), V(1, 1), K(2, 0), K(2, 1), V(2, 0), V(2, 1),
                    K(3, 0), K(3, 1), V(3, 0), V(3, 1), Q(1, 0), Q(1, 1)],
                1: [Q(2, 0), Q(2, 1)],
                2: [Q(3, 0), Q(3, 1)],
                3: [],
            }

            # ---- attention main loop ----
            # accA bank: cols s*128+h*64 = A(slice s, head h) [128,64]
            # dens bank: cols (n%2)*8 + s*2 + h = den column of chunk n
            prev = None          # (P, m, n) awaiting attn-out
            pending = []         # normalization chain tasks of chunk n-1

            def emit_attnout(pP, pm, pn):
                first = pm == 0
                dof = (pn % 2) * 8
                for s in range(4):
                    for h in range(2):
                        stat = pP[:, h * 512 + s * 128:h * 512 + (s + 1) * 128]
                        nc.tensor.matmul(
                            acc[:, s * 128 + h * 64:s * 128 + h * 64 + 64],
                            stat,
                            v2_all[:, pm * 128 + h * 64:pm * 128 + h * 64 + 64],
                            start=(first and s == 0 and h == 0),
                            stop=(pm == 15), skip_group_check=True)
                        nc.tensor.matmul(
                            dens[:, dof + s * 2 + h:dof + s * 2 + h + 1],
                            stat, ones1,
                            start=(first and s == 0 and h == 0),
                            stop=(pm == 15), skip_group_check=True)

            def make_pending(n):
                # release-copy acc -> sbuf, then engine-batched waves:
                # norms (Pool/DVE), transposes (PE) + copies (DVE),
                # outproj (PE) + so-copies (DVE; + ScalarE on final chunk).
                final = n == 3
                asb = npool.tile([128, 512], F32, tag="asb", name=f"{r}asb{n}")
                rc = npool.tile([128, 8], F32, tag="rc", name=f"{r}rc{n}")
                apk = [None] * 4
                attnT = npool.tile([128, 512], F16, tag="attnT",
                                   name=f"{r}attnT{n}")
                so = npool.tile([128, 2048], F16, tag="so", name=f"{r}so{n}")

                def release():
                    nc.vector.tensor_copy(asb, acc)
                    nc.vector.reciprocal(rc, dens[:, (n % 2) * 8:(n % 2) * 8 + 8])

                def norms():
                    for s in range(4):
                        t = apool.tile([128, 128], F16, tag=f"apk{s % 2}",
                                       name=f"{r}apk{n}_{s}")
                        eng = nc.vector if (final and s % 2) else nc.gpsimd
                        for h in range(2):
                            eng.tensor_scalar(
                                t[:, h * 64:(h + 1) * 64],
                                asb[:, s * 128 + h * 64:s * 128 + h * 64 + 64],
                                rc[:, s * 2 + h:s * 2 + h + 1], None, MULT)
                        apk[s] = t

                def trans(s0):
                    pstt = psS.tile([128, 1024], F32, tag="blk",
                                    name=f"{r}pst{n}_{s0}")
                    pv = pstt[:, 0:512].bitcast(F16)
                    for i, s in enumerate((s0, s0 + 1)):
                        nc.tensor.transpose(pv[:, i * 128:(i + 1) * 128],
                                            apk[s], ident16)
                    for i, s in enumerate((s0, s0 + 1)):
                        nc.vector.tensor_copy(attnT[:, s * 128:(s + 1) * 128],
                                              pv[:, i * 128:(i + 1) * 128])

                def outproj(s):
                    pot = psS.tile([128, 1024], F32, tag="blk",
                                   name=f"{r}po{n}_{s}")
                    po = pot[:, 0:512]
                    for cc in range(4):
                        nc.tensor.matmul(po[:, cc * 128:(cc + 1) * 128],
                                         wp_sb[:, cc * 128:(cc + 1) * 128],
                                         attnT[:, s * 128:(s + 1) * 128],
                                         start=True, stop=True)
                    sos = so[:, s * 512:(s + 1) * 512]
                    if final and s % 2 == 0:
                        nc.scalar.activation(sos, po, COPY)
                    else:
                        nc.vector.tensor_copy(sos, po)
                    nc.sync.dma_start(
                        out=outT.rearrange("(cc p) n -> p cc n", p=128)[
                            :, :, n * 512 + s * 128:n * 512 + (s + 1) * 128],
                        in_=sos)

                # None = deliberate spacing: PE work in a task stalls the
                # whole in-order PE stream if its DVE/Pool inputs lag, so
                # give each producer a couple of steps of lead time.
                return [release, norms, None, lambda: trans(0),
                        lambda: trans(2), None, lambda: outproj(0),
                        lambda: outproj(1), None, lambda: outproj(2),
                        lambda: outproj(3)]

            # global 64-step stream; scores are issued ONE step ahead of
            # their exp so the PE keeps ScalarE fed across engine switches
            # and chunk boundaries.
            steps = [(n, m) for n in range(4) for m in range(16)]
            blks = {}

            def emit_scores(n, m):
                ns = slice(n * 512, (n + 1) * 512)
                msl = slice(m * 128, (m + 1) * 128)
                blk = psS.tile([128, 1024], F32, tag="blk",
                               name=f"{r}blk{n}_{m}")
                nc.tensor.matmul(blk[:, 0:512], kT[0:64, msl],
                                 qT[0:64, ns], start=True, stop=True,
                                 tile_position=(0, 0))
                nc.tensor.matmul(blk[:, 512:1024], kT[64:128, msl],
                                 qT[64:128, ns], start=True, stop=True,
                                 tile_position=(64, 0))
                blks[(n, m)] = blk

            tasks = []
            emit_scores(0, 0)
            emit_scores(0, 1)
            v_proj(0, 0, hold)
            v_proj(0, 1, hold)
            kq_proj(kT, wk_sb, y_all, 1, 0, hold, f"{r}psk1")
            kq_proj(kT, wk_sb, y_all, 1, 1, hold, f"{r}psk1")
            for si, (n, m) in enumerate(steps):
                if m == 0:
                    tasks = pending + fills[n]
                    pending = []
                if 1 <= si < len(steps) - 1:
                    emit_scores(*steps[si + 1])
                blk = blks.pop((n, m))
                P = ppool.tile([128, 1024], F16, tag="p", name=f"{r}p{n}_{m}")
                if m in DVE_EXP_M[n]:
                    nc.vector.tensor_scalar(P.bitcast(I16), blk,
                                            FE_SCALE, FE_BIAS, MULT, ADD)
                else:
                    nc.scalar.activation(P, blk, EXP)
                if m >= 1 and tasks:
                    task = tasks.pop(0)
                    if task is not None:
                        task()
                if prev is not None:
                    emit_attnout(*prev)
                prev = (P, m, n)
                if m == 15:
                    pending = make_pending(n)

            # drain: last attn-out + final chunk's normalization chain
            emit_attnout(*prev)
            prev = None
            for task in pending:
                if task is not None:
                    task()

    nc.compile()
    return nc


def _get_program():
    global _NC
    if _NC is None:
        _NC = _build_program()
    return _NC


def make_in_maps(inputs):
    x = np.asarray(inputs["x"], np.float32)
    y = np.asarray(inputs["y"], np.float32)
    Wq = np.asarray(inputs["Wq"], np.float32)
    Wkv = np.asarray(inputs["Wkv"], np.float32)
    lw = np.asarray(inputs["lw"], np.float32)

    d = np.arange(HD)
    xr = [x[b].astype(np.float16) for b in range(B)]
    yr = [y[b].astype(np.float16) for b in range(B)]
    in_maps = []
    for core in range(NCORES):
        b = core // 4
        h0 = (core % 4) * 2
        ch = np.concatenate([h * HD + d for h in (h0, h0 + 1)])  # channels
        colsK = np.concatenate([h * 2 * HD + 2 * d for h in (h0, h0 + 1)])
        wq_c = Wq[:, ch] * np.float32(SCALE)
        wk_c = Wkv[:, colsK]
        wv_c = Wkv[:, colsK + 1] * (1.0 + lw[ch])[None, :]
        wp_c = np.asarray(inputs["Wp"], np.float32)[ch, :]
        in_maps.append({
            "xr": xr[b],
            "yr": yr[b],
            "wq": wq_c.astype(np.float16),
            "wk": wk_c.astype(np.float16),
            "wv": wv_c.astype(np.float16),
            "wp": wp_c.astype(np.float16),
        })
    return in_maps


def assemble_output(results, inputs):
    lb = np.asarray(inputs["lb"], np.float32)
    Wp = np.asarray(inputs["Wp"], np.float32)
    bp = np.asarray(inputs["bp"], np.float32)
    bias = (bp + lb @ Wp).astype(np.float32)
    out = np.stack([
        sum(results[i]["outT"].astype(np.float32) for i in range(4)),
        sum(results[i]["outT"].astype(np.float32) for i in range(4, 8)),
    ])
    out += bias[None, :, None]
    return out.astype(np.float32)


def kernel(x, y, Wq, Wkv, lw, lb, Wp, bp):
    global LAST_RUN
    from concourse.bass_utils import run_bass_kernel_spmd

    inputs = dict(x=x, y=y, Wq=Wq, Wkv=Wkv, lw=lw, lb=lb, Wp=Wp, bp=bp)
    nc = _get_program()
    in_maps = make_in_maps(inputs)
    LAST_RUN = run_bass_kernel_spmd(nc, in_maps, list(range(NCORES)))
    return assemble_output(LAST_RUN.results, inputs)
